# revision 12
# baseline (speedup 1.0000x reference)
"""GNN message-passing encoder on 8 Trainium2 NeuronCores.

Computation (see module docstring of the problem):
    h      = l2norm(relu(x @ W + b))                    [N, 128]
    neigh1 = segment_mean(h[src], dst)                  [N, 128]
    neigh2 = segment_mean(neigh1[src], dst)             [N, 128]
    out    = (h, 0.7*neigh1 + 0.3*neigh2)

Distribution: nodes are range-sharded across the 8 cores.  Each core runs the
MLP on its node shard, the fp16 feature table is AllGather'd (in two halves so
the collective overlaps with compute and so every gather index fits in int16),
and each core aggregates the edges whose dst it owns.  Edge aggregation is a
gather (dma_gather, batched per superblock) followed by a one-hot matmul
segment-sum into PSUM.  All graph index manipulation (bucketing, tiling,
degree computation) happens on the host; the device does all float math.
"""

import sys

for _p in ("/opt/trn_rl_repo",):
    if _p not in sys.path:
        sys.path.insert(0, _p)

import numpy as np

# ---------------------------------------------------------------- constants
N_NODES = 50000
N_EDGES = 800000
D_IN = 256
D_OUT = 128
NCORES = 8
LAM = 0.7
P = 128

NC_NODES = N_NODES // NCORES            # 6250 nodes per core
HALF = NC_NODES // 2                    # 3125: local-row split for the 2 AG halves
TAB_ROWS = NCORES * HALF                # 25000 rows per half-table (< 32768, int16-safe)
NB = (NC_NODES + P - 1) // P            # 49 dst blocks of 128 nodes per core
SBK = 7                                 # dst blocks per superblock (49 = 7*7)
NSB = (NB + SBK - 1) // SBK
AG_SPLIT_BLOCK = HALF // P              # block 24 finishes rows [0, HALF)

assert TAB_ROWS < 32768
STAGES = "full"  # debug: "mlp", "mlp+ag", "hop1", "full"


# ---------------------------------------------------------------- host prep
def _build_layout(src, dst):
    """Bucket/tile the edge list.  Returns the (core-uniform) tile layout and
    the per-core metadata arrays that parameterize the device program."""
    deg = np.bincount(dst, minlength=N_NODES).astype(np.float32)
    recip = (1.0 / np.maximum(deg, 1.0)).astype(np.float32)

    owner = dst // NC_NODES
    per_core = []
    for c in range(NCORES):
        sel = np.nonzero(owner == c)[0]
        e_dst = dst[sel] - c * NC_NODES
        e_src = src[sel]
        blk = e_dst >> 7
        dst_mod = (e_dst & 127).astype(np.float32)
        s_c = e_src // NC_NODES
        s_i = e_src % NC_NODES
        grp = (s_i >= HALF).astype(np.int8)
        tabidx = (s_c * HALF + np.where(grp, s_i - HALF, s_i)).astype(np.int16)
        w1 = recip[dst[sel]].astype(np.float32)
        w2 = ((1.0 - LAM) * recip[dst[sel]]).astype(np.float32)
        # bucket into (block, grp) lists
        buckets = {}
        order = np.lexsort((e_src, grp, blk))
        for g in (0, 1):
            for b in range(NB):
                buckets[(b, g)] = []
        bs = blk[order]
        gs = grp[order]
        for pos, ei in enumerate(order):
            buckets[(int(bs[pos]), int(gs[pos]))].append(ei)
        per_core.append(
            dict(buckets=buckets, tabidx=tabidx, dst_mod=dst_mod, w1=w1, w2=w2)
        )

    # core-uniform tile counts per (block, grp)
    TG = np.zeros((NB, 2), np.int64)
    for b in range(NB):
        for g in (0, 1):
            mx = max(len(per_core[c]["buckets"][(b, g)]) for c in range(NCORES))
            TG[b, g] = (mx + P - 1) // P
    # every block needs at least one tile so its PSUM group is well formed
    for b in range(NB):
        if TG[b].sum() == 0:
            TG[b, 0] = 1

    # enumerate superblocks / tiles / gather slabs
    sbs = []
    tau = 0
    ofsA = 0
    ofsB = 0
    for s in range(NSB):
        blocks = list(range(s * SBK, min((s + 1) * SBK, NB)))
        TaS = int(sum(TG[b, 0] for b in blocks))
        TbS = int(sum(TG[b, 1] for b in blocks))
        tiles = {b: [] for b in blocks}  # block -> [(tau, slot)]
        slot = 0
        for g in (0, 1):
            for b in blocks:
                for _t in range(int(TG[b, g])):
                    tiles[b].append((tau, slot))
                    tau += 1
                    slot += 1
        sbs.append(
            dict(
                blocks=blocks, TaS=TaS, TbS=TbS, nA=TaS * P, nB=TbS * P,
                ofsA=ofsA, ofsB=ofsB, tiles=tiles,
            )
        )
        ofsA += TaS * P // 16
        ofsB += TbS * P // 16
    T_total = tau
    SIA, SIB = ofsA, ofsB

    # per-core metadata arrays
    metas = []
    for c in range(NCORES):
        pc = per_core[c]
        m_dst = np.zeros((T_total, P), np.float32)
        m_w1 = np.zeros((T_total, P), np.float32)
        m_w2 = np.zeros((T_total, P), np.float32)
        idx_a = np.zeros((16, SIA), np.int16)
        idx_b = np.zeros((16, SIB), np.int16)
        for s in range(NSB):
            sb = sbs[s]
            for g, (idx_sl, ofs) in enumerate(
                ((idx_a, sb["ofsA"]), (idx_b, sb["ofsB"]))
            ):
                q = 0  # index position within this (superblock, grp) gather call
                for b in sb["blocks"]:
                    eids = pc["buckets"][(b, g)]
                    ntile = int(TG[b, g])
                    pad = ntile * P
                    for k in range(pad):
                        if k < len(eids):
                            ei = eids[k]
                            val = pc["tabidx"][ei]
                        else:
                            val = 0
                        idx_sl[q % 16, ofs + q // 16] = val
                        q += 1
            for b in sb["blocks"]:
                # tiles[b] lists A tiles then B tiles
                nA_t = int(TG[b, 0])
                for j, (tt, _slot) in enumerate(sb["tiles"][b]):
                    g = 0 if j < nA_t else 1
                    tloc = j if j < nA_t else j - nA_t
                    eids = pc["buckets"][(b, g)]
                    lo = tloc * P
                    for k in range(P):
                        if lo + k < len(eids):
                            ei = eids[lo + k]
                            m_dst[tt, k] = pc["dst_mod"][ei]
                            m_w1[tt, k] = pc["w1"][ei]
                            m_w2[tt, k] = pc["w2"][ei]
        metas.append(
            dict(
                m_dst=np.ascontiguousarray(m_dst.T),   # [128, T]
                m_w1=np.ascontiguousarray(m_w1.T),
                m_w2=np.ascontiguousarray(m_w2.T),
                idx_a=np.tile(idx_a, (8, 1)),          # [128, SIA]
                idx_b=np.tile(idx_b, (8, 1)),
            )
        )

    layout = dict(T=T_total, SIA=SIA, SIB=SIB, sbs=sbs, TG=TG)
    return layout, metas


def _layout_key(layout):
    key = [layout["T"], layout["SIA"], layout["SIB"]]
    for sb in layout["sbs"]:
        key += [sb["TaS"], sb["TbS"], sb["ofsA"], sb["ofsB"]]
        for b in sb["blocks"]:
            key.append(tuple(t for t, _ in sb["tiles"][b]))
            key.append(tuple(s for _, s in sb["tiles"][b]))
    return tuple(key)


# ---------------------------------------------------------------- device IR
_PROGRAM_CACHE = {}


def _build_program(layout):
    from contextlib import ExitStack

    import concourse.bacc as bacc
    import concourse.tile as tile
    from concourse import mybir
    from concourse.bass import _add_dep_helper
    from concourse.tile import TileContext

    f32 = mybir.dt.float32
    f16 = mybir.dt.float16
    i16 = mybir.dt.int16
    Alu = mybir.AluOpType
    Act = mybir.ActivationFunctionType

    T = layout["T"]
    SIA = layout["SIA"]
    SIB = layout["SIB"]
    sbs = layout["sbs"]

    nc = bacc.Bacc("TRN2", target_bir_lowering=False, debug=False,
                   num_devices=NCORES)

    # I/O
    xt_d = nc.dram_tensor("xt", [NSB, 2, P, SBK * P], f32, kind="ExternalInput")
    w_d = nc.dram_tensor("wmat", [2, P, D_OUT], f32, kind="ExternalInput")
    bias_d = nc.dram_tensor("bias", [1, D_OUT], f32, kind="ExternalInput")
    ones_d = nc.dram_tensor("ones1", [1, P], f32, kind="ExternalInput")
    iota_d = nc.dram_tensor("iota", [P, P], f32, kind="ExternalInput")
    mdst_d = nc.dram_tensor("m_dst", [P, T], f32, kind="ExternalInput")
    mw1_d = nc.dram_tensor("m_w1", [P, T], f32, kind="ExternalInput")
    mw2_d = nc.dram_tensor("m_w2", [P, T], f32, kind="ExternalInput")
    idxa_d = nc.dram_tensor("idx_a", [P, SIA], i16, kind="ExternalInput")
    idxb_d = nc.dram_tensor("idx_b", [P, SIB], i16, kind="ExternalInput")

    h_out_d = nc.dram_tensor("h_out", [NC_NODES, D_OUT], f32, kind="ExternalOutput")
    mh_out_d = nc.dram_tensor("mh_out", [NC_NODES, D_OUT], f32, kind="ExternalOutput")

    # internal DRAM
    hshard_d = nc.dram_tensor("hshard16", [NC_NODES, D_OUT], f16)
    n1shard_d = nc.dram_tensor("n1shard16", [NC_NODES, D_OUT], f16)
    htab_a = nc.dram_tensor("htab_a", [TAB_ROWS, D_OUT], f16, addr_space="Shared")
    htab_b = nc.dram_tensor("htab_b", [TAB_ROWS, D_OUT], f16, addr_space="Shared")
    ntab_a = nc.dram_tensor("ntab_a", [TAB_ROWS, D_OUT], f16, addr_space="Shared")
    ntab_b = nc.dram_tensor("ntab_b", [TAB_ROWS, D_OUT], f16, addr_space="Shared")

    rg = [list(range(NCORES))]

    with TileContext(nc) as tc, ExitStack() as ctx:
        const = ctx.enter_context(tc.tile_pool(name="const", bufs=1))
        meta = ctx.enter_context(tc.tile_pool(name="meta", bufs=1))
        xtp = ctx.enter_context(tc.tile_pool(name="xtp", bufs=2))
        featp = ctx.enter_context(tc.tile_pool(name="featp", bufs=2))
        accp = ctx.enter_context(tc.tile_pool(name="accp", bufs=1))
        work = ctx.enter_context(tc.tile_pool(name="work", bufs=3))
        ohp = ctx.enter_context(tc.tile_pool(name="ohp", bufs=6))
        outp = ctx.enter_context(tc.tile_pool(name="outp", bufs=3))
        psmlp = ctx.enter_context(tc.tile_pool(name="psmlp", bufs=3, space="PSUM"))
        pshop = ctx.enter_context(tc.tile_pool(name="pshop", bufs=4, space="PSUM"))

        # ---- constant / metadata loads
        iota_sb = const.tile([P, P], f32)
        nc.sync.dma_start(iota_sb[:], iota_d[:, :])
        w_sb = [const.tile([P, D_OUT], f32, tag=f"w{t}", name=f"w_sb{t}")
                for t in range(2)]
        for t in range(2):
            nc.sync.dma_start(w_sb[t][:], w_d[t])
        ones_sb = const.tile([1, P], f32, tag="ones")
        nc.sync.dma_start(ones_sb[:], ones_d[:, :])
        bias_sb = const.tile([1, D_OUT], f32, tag="bias")
        nc.sync.dma_start(bias_sb[:], bias_d[:, :])
        mdst_sb = meta.tile([P, T], f32, tag="mdst")
        nc.sync.dma_start(mdst_sb[:], mdst_d[:, :])
        mw1_sb = meta.tile([P, T], f32, tag="mw1")
        nc.sync.dma_start(mw1_sb[:], mw1_d[:, :])
        mw2_sb = meta.tile([P, T], f32, tag="mw2")
        nc.sync.dma_start(mw2_sb[:], mw2_d[:, :])
        idxa_sb = meta.tile([P, SIA], i16, tag="idxa")
        nc.sync.dma_start(idxa_sb[:], idxa_d[:, :])
        idxb_sb = meta.tile([P, SIB], i16, tag="idxb")
        nc.sync.dma_start(idxb_sb[:], idxb_d[:, :])

        acc_n1 = accp.tile([P, NB * D_OUT], f32, tag="accn1")

        # ---- phase 1: MLP  h = l2norm(relu(x @ W + b))
        ag_insts = {}

        def emit_ag(name, src_ap, dst_ap):
            inst = nc.gpsimd.collective_compute(
                "AllGather", Alu.bypass, replica_groups=rg,
                ins=[src_ap], outs=[dst_ap],
            )
            ag_insts[name] = inst
            return inst

        for s in range(NSB):
            xts = xtp.tile([P, 2, SBK * P], f32, tag="xts")
            for t in range(2):
                nc.sync.dma_start(xts[:, t, :], xt_d[s, t])
            for bl in range(SBK):
                B = s * SBK + bl
                if B >= NB:
                    break
                ps = psmlp.tile([P, D_OUT], f32, tag="psmlp")
                for t in range(2):
                    nc.tensor.matmul(
                        ps[:], lhsT=xts[:, t, bl * P:(bl + 1) * P],
                        rhs=w_sb[t][:], start=(t == 0), stop=False,
                    )
                nc.tensor.matmul(ps[:], lhsT=ones_sb[:], rhs=bias_sb[:],
                                 start=False, stop=True)
                hb = work.tile([P, D_OUT], f32, tag="hb")
                nc.scalar.activation(hb[:], ps[:], Act.Relu)
                sq = work.tile([P, D_OUT], f32, tag="sq")
                ns = work.tile([P, 1], f32, tag="ns")
                nc.scalar.activation(sq[:], hb[:], Act.Square, accum_out=ns[:])
                nsc = work.tile([P, 1], f32, tag="nsc")
                nc.vector.tensor_scalar(out=nsc[:], in0=ns[:], scalar1=1e-24,
                                        scalar2=None, op0=Alu.max)
                sqr = work.tile([P, 1], f32, tag="sqr")
                nc.scalar.activation(sqr[:], nsc[:], Act.Sqrt)
                rn = work.tile([P, 1], f32, tag="rn")
                nc.vector.reciprocal(rn[:], sqr[:])
                hO = outp.tile([P, D_OUT], f32, tag="hO")
                nc.scalar.activation(hO[:], hb[:], Act.Copy, scale=rn[:])
                h16 = outp.tile([P, D_OUT], f16, tag="h16")
                nc.vector.tensor_copy(h16[:], hO[:])
                rows = min(P, NC_NODES - B * P)
                nc.sync.dma_start(h_out_d[B * P:B * P + rows, :], hO[:rows, :])
                nc.sync.dma_start(hshard_d[B * P:B * P + rows, :], h16[:rows, :])
                if B == AG_SPLIT_BLOCK and STAGES != "mlp":
                    emit_ag("h_a", hshard_d[0:HALF, :], htab_a[:, :])
        if STAGES != "mlp":
            emit_ag("h_b", hshard_d[HALF:NC_NODES, :], htab_b[:, :])

        # ---- phases 2/3: the two aggregation hops
        def emit_hop(tab_a, tab_b, w_meta, dep_a, dep_b, flush):
            for s in range(NSB):
                sb = sbs[s]
                TS = sb["TaS"] + sb["TbS"]
                fb = featp.tile([P, TS, D_OUT], f16, tag="fb")
                if sb["nA"] > 0:
                    gi = nc.gpsimd.dma_gather(
                        fb[:, 0:sb["TaS"], :], tab_a[:, :],
                        idxa_sb[:, sb["ofsA"]:sb["ofsA"] + sb["nA"] // 16],
                        sb["nA"], sb["nA"], D_OUT, single_packet=False,
                    )
                    _add_dep_helper(gi.ins, dep_a.ins, True, "gather after AG a")
                if sb["nB"] > 0:
                    gi = nc.gpsimd.dma_gather(
                        fb[:, sb["TaS"]:TS, :], tab_b[:, :],
                        idxb_sb[:, sb["ofsB"]:sb["ofsB"] + sb["nB"] // 16],
                        sb["nB"], sb["nB"], D_OUT, single_packet=False,
                    )
                    _add_dep_helper(gi.ins, dep_b.ins, True, "gather after AG b")
                for b in sb["blocks"]:
                    tl = sb["tiles"][b]
                    ps = pshop.tile([P, D_OUT], f32, tag="pshop")
                    for i, (tt, slot) in enumerate(tl):
                        oh = ohp.tile([P, P], f16, tag="oh")
                        nc.vector.tensor_scalar(
                            out=oh[:], in0=iota_sb[:],
                            scalar1=mdst_sb[:, tt:tt + 1],
                            scalar2=w_meta[:, tt:tt + 1],
                            op0=Alu.is_equal, op1=Alu.mult,
                        )
                        nc.tensor.matmul(ps[:], lhsT=oh[:], rhs=fb[:, slot, :],
                                         start=(i == 0), stop=(i == len(tl) - 1))
                    flush(b, ps)

        if STAGES in ("mlp", "mlp+ag"):
            nc.compile_hook_skip_hops = True

        def flush1(B, ps):
            nc.scalar.activation(acc_n1[:, B * D_OUT:(B + 1) * D_OUT], ps[:],
                                 Act.Copy)
            n16 = outp.tile([P, D_OUT], f16, tag="n16")
            nc.vector.tensor_copy(n16[:], ps[:])
            rows = min(P, NC_NODES - B * P)
            nc.sync.dma_start(n1shard_d[B * P:B * P + rows, :], n16[:rows, :])
            if B == AG_SPLIT_BLOCK and STAGES != "hop1":
                emit_ag("n_a", n1shard_d[0:HALF, :], ntab_a[:, :])

        if STAGES not in ("mlp", "mlp+ag"):
            emit_hop(htab_a, htab_b, mw1_sb, ag_insts["h_a"], ag_insts["h_b"],
                     flush1)
        if STAGES == "full":
            emit_ag("n_b", n1shard_d[HALF:NC_NODES, :], ntab_b[:, :])

        def flush2(B, ps):
            mh = outp.tile([P, D_OUT], f32, tag="mh")
            nc.vector.scalar_tensor_tensor(
                out=mh[:], in0=acc_n1[:, B * D_OUT:(B + 1) * D_OUT],
                scalar=LAM, in1=ps[:], op0=Alu.mult, op1=Alu.add,
            )
            rows = min(P, NC_NODES - B * P)
            nc.sync.dma_start(mh_out_d[B * P:B * P + rows, :], mh[:rows, :])

        if STAGES == "full":
            emit_hop(ntab_a, ntab_b, mw2_sb, ag_insts["n_a"], ag_insts["n_b"],
                     flush2)

    nc.compile()
    return nc


# ---------------------------------------------------------------- entry
def _build_in_maps(x, W, b, metas):
    iota = np.tile(np.arange(P, dtype=np.float32), (P, 1))
    wmat = np.stack([W[0:P, :], W[P:2 * P, :]]).astype(np.float32)
    bias = b.reshape(1, D_OUT).astype(np.float32)
    ones1 = np.ones((1, P), np.float32)

    in_maps = []
    for c in range(NCORES):
        xs = x[c * NC_NODES:(c + 1) * NC_NODES]
        xs_pad = np.zeros((NSB * SBK * P, D_IN), np.float32)
        xs_pad[:NC_NODES] = xs
        xt = np.zeros((NSB, 2, P, SBK * P), np.float32)
        for s in range(NSB):
            chunk = xs_pad[s * SBK * P:(s + 1) * SBK * P]  # [896, 256]
            ct = np.ascontiguousarray(chunk.T)             # [256, 896]
            xt[s, 0] = ct[0:P]
            xt[s, 1] = ct[P:2 * P]
        m = metas[c]
        in_maps.append(
            dict(
                xt=xt, wmat=wmat, bias=bias, ones1=ones1, iota=iota,
                m_dst=m["m_dst"], m_w1=m["m_w1"], m_w2=m["m_w2"],
                idx_a=m["idx_a"], idx_b=m["idx_b"],
            )
        )
    return in_maps


def kernel(x, W, b, src, dst):
    x = np.asarray(x, np.float32)
    W = np.asarray(W, np.float32)
    b = np.asarray(b, np.float32)
    src = np.asarray(src, np.int32)
    dst = np.asarray(dst, np.int32)

    layout, metas = _build_layout(src, dst)
    key = _layout_key(layout)
    if key not in _PROGRAM_CACHE:
        _PROGRAM_CACHE[key] = _build_program(layout)
    nc = _PROGRAM_CACHE[key]
    in_maps = _build_in_maps(x, W, b, metas)

    from concourse.bass_utils import run_bass_kernel_spmd

    res = run_bass_kernel_spmd(nc, in_maps, list(range(NCORES)))
    h = np.concatenate([res.results[c]["h_out"] for c in range(NCORES)], axis=0)
    mh = np.concatenate([res.results[c]["mh_out"] for c in range(NCORES)], axis=0)
    return (h, mh)


# revision 13
# speedup vs baseline: 1.6716x; 1.6716x over previous
"""GNN message-passing encoder on 8 Trainium2 NeuronCores.

Computation (see module docstring of the problem):
    h      = l2norm(relu(x @ W + b))                    [N, 128]
    neigh1 = segment_mean(h[src], dst)                  [N, 128]
    neigh2 = segment_mean(neigh1[src], dst)             [N, 128]
    out    = (h, 0.7*neigh1 + 0.3*neigh2)

Distribution: nodes are range-sharded across the 8 cores.  Each core runs the
MLP on its node shard, the fp16 feature table is AllGather'd (in two halves so
the collective overlaps with compute and so every gather index fits in int16),
and each core aggregates the edges whose dst it owns.  Edge aggregation is a
gather (dma_gather, batched per superblock) followed by a one-hot matmul
segment-sum into PSUM.  All graph index manipulation (bucketing, tiling,
degree computation) happens on the host; the device does all float math.
"""

import sys

for _p in ("/opt/trn_rl_repo",):
    if _p not in sys.path:
        sys.path.insert(0, _p)

import numpy as np

# ---------------------------------------------------------------- constants
N_NODES = 50000
N_EDGES = 800000
D_IN = 256
D_OUT = 128
NCORES = 8
LAM = 0.7
P = 128

NC_NODES = N_NODES // NCORES            # 6250 nodes per core
HALF = NC_NODES // 2                    # 3125: local-row split for the 2 AG halves
TAB_ROWS = NCORES * HALF                # 25000 rows per half-table (< 32768, int16-safe)
NB = (NC_NODES + P - 1) // P            # 49 dst blocks of 128 nodes per core
SBK = 7                                 # dst blocks per superblock (49 = 7*7)
NSB = (NB + SBK - 1) // SBK
AG_SPLIT_BLOCK = HALF // P              # block 24 finishes rows [0, HALF)

assert TAB_ROWS < 32768
STAGES = "full"  # debug: "mlp", "mlp+ag", "hop1", "full"


# ---------------------------------------------------------------- host prep
def _build_layout(src, dst):
    """Bucket/tile the edge list.  Returns the (core-uniform) tile layout and
    the per-core metadata arrays that parameterize the device program."""
    deg = np.bincount(dst, minlength=N_NODES).astype(np.float32)
    recip = (1.0 / np.maximum(deg, 1.0)).astype(np.float32)

    owner = dst // NC_NODES
    per_core = []
    for c in range(NCORES):
        sel = np.nonzero(owner == c)[0]
        e_dst = dst[sel] - c * NC_NODES
        e_src = src[sel]
        blk = e_dst >> 7
        dst_mod = (e_dst & 127).astype(np.float32)
        s_c = e_src // NC_NODES
        s_i = e_src % NC_NODES
        grp = (s_i >= HALF).astype(np.int8)
        tabidx = (s_c * HALF + np.where(grp, s_i - HALF, s_i)).astype(np.int16)
        w1 = recip[dst[sel]].astype(np.float32)
        w2 = ((1.0 - LAM) * recip[dst[sel]]).astype(np.float32)
        # bucket into (block, grp) lists
        buckets = {}
        order = np.lexsort((e_src, grp, blk))
        for g in (0, 1):
            for b in range(NB):
                buckets[(b, g)] = []
        bs = blk[order]
        gs = grp[order]
        for pos, ei in enumerate(order):
            buckets[(int(bs[pos]), int(gs[pos]))].append(ei)
        per_core.append(
            dict(buckets=buckets, tabidx=tabidx, dst_mod=dst_mod, w1=w1, w2=w2)
        )

    # core-uniform tile counts per (block, grp)
    TG = np.zeros((NB, 2), np.int64)
    for b in range(NB):
        for g in (0, 1):
            mx = max(len(per_core[c]["buckets"][(b, g)]) for c in range(NCORES))
            TG[b, g] = (mx + P - 1) // P
    # every block needs at least one tile so its PSUM group is well formed
    for b in range(NB):
        if TG[b].sum() == 0:
            TG[b, 0] = 1

    # enumerate superblocks / tiles / gather slabs
    sbs = []
    tau = 0
    ofsA = 0
    ofsB = 0
    for s in range(NSB):
        blocks = list(range(s * SBK, min((s + 1) * SBK, NB)))
        TaS = int(sum(TG[b, 0] for b in blocks))
        TbS = int(sum(TG[b, 1] for b in blocks))
        tiles = {b: [] for b in blocks}  # block -> [(tau, slot)]
        slot = 0
        for g in (0, 1):
            for b in blocks:
                for _t in range(int(TG[b, g])):
                    tiles[b].append((tau, slot))
                    tau += 1
                    slot += 1
        sbs.append(
            dict(
                blocks=blocks, TaS=TaS, TbS=TbS, nA=TaS * P, nB=TbS * P,
                ofsA=ofsA, ofsB=ofsB, tiles=tiles,
            )
        )
        ofsA += TaS * P // 16
        ofsB += TbS * P // 16
    T_total = tau
    SIA, SIB = ofsA, ofsB

    # per-core metadata arrays
    metas = []
    for c in range(NCORES):
        pc = per_core[c]
        m_dst = np.zeros((T_total, P), np.float32)
        m_w1 = np.zeros((T_total, P), np.float32)
        m_w2 = np.zeros((T_total, P), np.float32)
        idx_a = np.zeros((16, SIA), np.int16)
        idx_b = np.zeros((16, SIB), np.int16)
        for s in range(NSB):
            sb = sbs[s]
            for g, (idx_sl, ofs) in enumerate(
                ((idx_a, sb["ofsA"]), (idx_b, sb["ofsB"]))
            ):
                q = 0  # index position within this (superblock, grp) gather call
                for b in sb["blocks"]:
                    eids = pc["buckets"][(b, g)]
                    ntile = int(TG[b, g])
                    pad = ntile * P
                    for k in range(pad):
                        if k < len(eids):
                            ei = eids[k]
                            val = pc["tabidx"][ei]
                        else:
                            val = 0
                        idx_sl[q % 16, ofs + q // 16] = val
                        q += 1
            for b in sb["blocks"]:
                # tiles[b] lists A tiles then B tiles
                nA_t = int(TG[b, 0])
                for j, (tt, _slot) in enumerate(sb["tiles"][b]):
                    g = 0 if j < nA_t else 1
                    tloc = j if j < nA_t else j - nA_t
                    eids = pc["buckets"][(b, g)]
                    lo = tloc * P
                    for k in range(P):
                        if lo + k < len(eids):
                            ei = eids[lo + k]
                            m_dst[tt, k] = pc["dst_mod"][ei]
                            m_w1[tt, k] = pc["w1"][ei]
                            m_w2[tt, k] = pc["w2"][ei]
        metas.append(
            dict(
                m_dst=np.ascontiguousarray(m_dst.T),   # [128, T]
                m_w1=np.ascontiguousarray(m_w1.T),
                m_w2=np.ascontiguousarray(m_w2.T),
                idx_a=np.tile(idx_a, (8, 1)),          # [128, SIA]
                idx_b=np.tile(idx_b, (8, 1)),
            )
        )

    layout = dict(T=T_total, SIA=SIA, SIB=SIB, sbs=sbs, TG=TG)
    return layout, metas


def _layout_key(layout):
    key = [layout["T"], layout["SIA"], layout["SIB"]]
    for sb in layout["sbs"]:
        key += [sb["TaS"], sb["TbS"], sb["ofsA"], sb["ofsB"]]
        for b in sb["blocks"]:
            key.append(tuple(t for t, _ in sb["tiles"][b]))
            key.append(tuple(s for _, s in sb["tiles"][b]))
    return tuple(key)


# ---------------------------------------------------------------- device IR
_PROGRAM_CACHE = {}


def _build_program(layout):
    from contextlib import ExitStack

    import concourse.bacc as bacc
    import concourse.tile as tile
    from concourse import mybir
    from concourse.bass import _add_dep_helper
    from concourse.tile import TileContext

    f32 = mybir.dt.float32
    f16 = mybir.dt.float16
    i16 = mybir.dt.int16
    Alu = mybir.AluOpType
    Act = mybir.ActivationFunctionType

    T = layout["T"]
    SIA = layout["SIA"]
    SIB = layout["SIB"]
    sbs = layout["sbs"]

    nc = bacc.Bacc("TRN2", target_bir_lowering=False, debug=False,
                   num_devices=NCORES, num_swdge_queues=4)

    # I/O
    xt_d = nc.dram_tensor("xt", [NSB, 2, P, SBK * P], f32, kind="ExternalInput")
    w_d = nc.dram_tensor("wmat", [2, P, D_OUT], f32, kind="ExternalInput")
    bias_d = nc.dram_tensor("bias", [1, D_OUT], f32, kind="ExternalInput")
    ones_d = nc.dram_tensor("ones1", [1, P], f32, kind="ExternalInput")
    iota_d = nc.dram_tensor("iota", [P, P], f32, kind="ExternalInput")
    mdst_d = nc.dram_tensor("m_dst", [P, T], f32, kind="ExternalInput")
    mw1_d = nc.dram_tensor("m_w1", [P, T], f32, kind="ExternalInput")
    mw2_d = nc.dram_tensor("m_w2", [P, T], f32, kind="ExternalInput")
    idxa_d = nc.dram_tensor("idx_a", [P, SIA], i16, kind="ExternalInput")
    idxb_d = nc.dram_tensor("idx_b", [P, SIB], i16, kind="ExternalInput")

    h_out_d = nc.dram_tensor("h_out", [NC_NODES, D_OUT], f32, kind="ExternalOutput")
    mh_out_d = nc.dram_tensor("mh_out", [NC_NODES, D_OUT], f32, kind="ExternalOutput")

    # internal DRAM
    hshard_d = nc.dram_tensor("hshard16", [NC_NODES, D_OUT], f16)
    n1shard_d = nc.dram_tensor("n1shard16", [NC_NODES, D_OUT], f16)
    htab_a = nc.dram_tensor("htab_a", [TAB_ROWS, D_OUT], f16, addr_space="Shared")
    htab_b = nc.dram_tensor("htab_b", [TAB_ROWS, D_OUT], f16, addr_space="Shared")
    ntab_a = nc.dram_tensor("ntab_a", [TAB_ROWS, D_OUT], f16, addr_space="Shared")
    ntab_b = nc.dram_tensor("ntab_b", [TAB_ROWS, D_OUT], f16, addr_space="Shared")

    rg = [list(range(NCORES))]

    with TileContext(nc) as tc, ExitStack() as ctx:
        const = ctx.enter_context(tc.tile_pool(name="const", bufs=1))
        meta = ctx.enter_context(tc.tile_pool(name="meta", bufs=1))
        xtp = ctx.enter_context(tc.tile_pool(name="xtp", bufs=2))
        featp = ctx.enter_context(tc.tile_pool(name="featp", bufs=2))
        accp = ctx.enter_context(tc.tile_pool(name="accp", bufs=1))
        work = ctx.enter_context(tc.tile_pool(name="work", bufs=3))
        ohp = ctx.enter_context(tc.tile_pool(name="ohp", bufs=6))
        outp = ctx.enter_context(tc.tile_pool(name="outp", bufs=3))
        psmlp = ctx.enter_context(tc.tile_pool(name="psmlp", bufs=3, space="PSUM"))
        pshop = ctx.enter_context(tc.tile_pool(name="pshop", bufs=4, space="PSUM"))

        # ---- constant / metadata loads
        iota_sb = const.tile([P, P], f32)
        nc.sync.dma_start(iota_sb[:], iota_d[:, :])
        w_sb = [const.tile([P, D_OUT], f32, tag=f"w{t}", name=f"w_sb{t}")
                for t in range(2)]
        for t in range(2):
            nc.sync.dma_start(w_sb[t][:], w_d[t])
        ones_sb = const.tile([1, P], f32, tag="ones")
        nc.sync.dma_start(ones_sb[:], ones_d[:, :])
        bias_sb = const.tile([1, D_OUT], f32, tag="bias")
        nc.sync.dma_start(bias_sb[:], bias_d[:, :])
        mdst_sb = meta.tile([P, T], f32, tag="mdst")
        nc.sync.dma_start(mdst_sb[:], mdst_d[:, :])
        mw1_sb = meta.tile([P, T], f32, tag="mw1")
        nc.sync.dma_start(mw1_sb[:], mw1_d[:, :])
        mw2_sb = meta.tile([P, T], f32, tag="mw2")
        nc.sync.dma_start(mw2_sb[:], mw2_d[:, :])
        idxa_sb = meta.tile([P, SIA], i16, tag="idxa")
        nc.sync.dma_start(idxa_sb[:], idxa_d[:, :])
        idxb_sb = meta.tile([P, SIB], i16, tag="idxb")
        nc.sync.dma_start(idxb_sb[:], idxb_d[:, :])

        acc_n1 = accp.tile([P, NB * D_OUT], f32, tag="accn1")

        # ---- phase 1: MLP  h = l2norm(relu(x @ W + b))
        ag_insts = {}

        def emit_ag(name, src_ap, dst_ap):
            inst = nc.gpsimd.collective_compute(
                "AllGather", Alu.bypass, replica_groups=rg,
                ins=[src_ap], outs=[dst_ap],
            )
            ag_insts[name] = inst
            return inst

        for s in range(NSB):
            xts = xtp.tile([P, 2, SBK * P], f32, tag="xts")
            for t in range(2):
                nc.sync.dma_start(xts[:, t, :], xt_d[s, t])
            for bl in range(SBK):
                B = s * SBK + bl
                if B >= NB:
                    break
                ps = psmlp.tile([P, D_OUT], f32, tag="psmlp")
                for t in range(2):
                    nc.tensor.matmul(
                        ps[:], lhsT=xts[:, t, bl * P:(bl + 1) * P],
                        rhs=w_sb[t][:], start=(t == 0), stop=False,
                    )
                nc.tensor.matmul(ps[:], lhsT=ones_sb[:], rhs=bias_sb[:],
                                 start=False, stop=True)
                hb = work.tile([P, D_OUT], f32, tag="hb")
                nc.scalar.activation(hb[:], ps[:], Act.Relu)
                sq = work.tile([P, D_OUT], f32, tag="sq")
                ns = work.tile([P, 1], f32, tag="ns")
                nc.scalar.activation(sq[:], hb[:], Act.Square, accum_out=ns[:])
                nsc = work.tile([P, 1], f32, tag="nsc")
                nc.vector.tensor_scalar(out=nsc[:], in0=ns[:], scalar1=1e-24,
                                        scalar2=None, op0=Alu.max)
                sqr = work.tile([P, 1], f32, tag="sqr")
                nc.scalar.activation(sqr[:], nsc[:], Act.Sqrt)
                rn = work.tile([P, 1], f32, tag="rn")
                nc.vector.reciprocal(rn[:], sqr[:])
                hO = outp.tile([P, D_OUT], f32, tag="hO")
                nc.scalar.activation(hO[:], hb[:], Act.Copy, scale=rn[:])
                h16 = outp.tile([P, D_OUT], f16, tag="h16")
                nc.vector.tensor_copy(h16[:], hO[:])
                rows = min(P, NC_NODES - B * P)
                nc.sync.dma_start(h_out_d[B * P:B * P + rows, :], hO[:rows, :])
                nc.sync.dma_start(hshard_d[B * P:B * P + rows, :], h16[:rows, :])
                if B == AG_SPLIT_BLOCK and STAGES != "mlp":
                    emit_ag("h_a", hshard_d[0:HALF, :], htab_a[:, :])
        if STAGES != "mlp":
            emit_ag("h_b", hshard_d[HALF:NC_NODES, :], htab_b[:, :])

        # ---- phases 2/3: the two aggregation hops
        qctr = [0]

        def emit_gather(fb, slot0, ntiles, tab, idx_sb, col0, dep, why):
            # split into two sub-calls on rotating SWDGE queues so several
            # DMA engines drain gathers concurrently
            for t0, t1 in ((0, ntiles // 2), (ntiles // 2, ntiles)):
                if t1 <= t0:
                    continue
                n = (t1 - t0) * P
                gi = nc.gpsimd.dma_gather(
                    fb[:, slot0 + t0:slot0 + t1, :], tab[:, :],
                    idx_sb[:, col0 + t0 * 8:col0 + t1 * 8],
                    n, n, D_OUT, single_packet=False,
                    queue_num=qctr[0] % 4,
                )
                qctr[0] += 1
                _add_dep_helper(gi.ins, dep.ins, True, why)

        def emit_hop(tab_a, tab_b, w_meta, dep_a, dep_b, flush):
            for s in range(NSB):
                sb = sbs[s]
                TS = sb["TaS"] + sb["TbS"]
                fb = featp.tile([P, TS, D_OUT], f16, tag="fb")
                if sb["nA"] > 0:
                    emit_gather(fb, 0, sb["TaS"], tab_a, idxa_sb, sb["ofsA"],
                                dep_a, "gather after AG a")
                if sb["nB"] > 0:
                    emit_gather(fb, sb["TaS"], sb["TbS"], tab_b, idxb_sb,
                                sb["ofsB"], dep_b, "gather after AG b")
                for b in sb["blocks"]:
                    tl = sb["tiles"][b]
                    ps = pshop.tile([P, D_OUT], f32, tag="pshop")
                    for i, (tt, slot) in enumerate(tl):
                        oh = ohp.tile([P, P], f16, tag="oh")
                        nc.vector.tensor_scalar(
                            out=oh[:], in0=iota_sb[:],
                            scalar1=mdst_sb[:, tt:tt + 1],
                            scalar2=w_meta[:, tt:tt + 1],
                            op0=Alu.is_equal, op1=Alu.mult,
                        )
                        nc.tensor.matmul(ps[:], lhsT=oh[:], rhs=fb[:, slot, :],
                                         start=(i == 0), stop=(i == len(tl) - 1))
                    flush(b, ps)

        if STAGES in ("mlp", "mlp+ag"):
            nc.compile_hook_skip_hops = True

        def flush1(B, ps):
            nc.scalar.activation(acc_n1[:, B * D_OUT:(B + 1) * D_OUT], ps[:],
                                 Act.Copy)
            n16 = outp.tile([P, D_OUT], f16, tag="n16")
            nc.vector.tensor_copy(n16[:], ps[:])
            rows = min(P, NC_NODES - B * P)
            nc.sync.dma_start(n1shard_d[B * P:B * P + rows, :], n16[:rows, :])
            if B == AG_SPLIT_BLOCK and STAGES != "hop1":
                emit_ag("n_a", n1shard_d[0:HALF, :], ntab_a[:, :])

        if STAGES not in ("mlp", "mlp+ag"):
            emit_hop(htab_a, htab_b, mw1_sb, ag_insts["h_a"], ag_insts["h_b"],
                     flush1)
        if STAGES == "full":
            emit_ag("n_b", n1shard_d[HALF:NC_NODES, :], ntab_b[:, :])

        def flush2(B, ps):
            mh = outp.tile([P, D_OUT], f32, tag="mh")
            nc.vector.scalar_tensor_tensor(
                out=mh[:], in0=acc_n1[:, B * D_OUT:(B + 1) * D_OUT],
                scalar=LAM, in1=ps[:], op0=Alu.mult, op1=Alu.add,
            )
            rows = min(P, NC_NODES - B * P)
            nc.sync.dma_start(mh_out_d[B * P:B * P + rows, :], mh[:rows, :])

        if STAGES == "full":
            emit_hop(ntab_a, ntab_b, mw2_sb, ag_insts["n_a"], ag_insts["n_b"],
                     flush2)

    nc.compile()
    return nc


# ---------------------------------------------------------------- entry
def _build_in_maps(x, W, b, metas):
    iota = np.tile(np.arange(P, dtype=np.float32), (P, 1))
    wmat = np.stack([W[0:P, :], W[P:2 * P, :]]).astype(np.float32)
    bias = b.reshape(1, D_OUT).astype(np.float32)
    ones1 = np.ones((1, P), np.float32)

    in_maps = []
    for c in range(NCORES):
        xs = x[c * NC_NODES:(c + 1) * NC_NODES]
        xs_pad = np.zeros((NSB * SBK * P, D_IN), np.float32)
        xs_pad[:NC_NODES] = xs
        xt = np.zeros((NSB, 2, P, SBK * P), np.float32)
        for s in range(NSB):
            chunk = xs_pad[s * SBK * P:(s + 1) * SBK * P]  # [896, 256]
            ct = np.ascontiguousarray(chunk.T)             # [256, 896]
            xt[s, 0] = ct[0:P]
            xt[s, 1] = ct[P:2 * P]
        m = metas[c]
        in_maps.append(
            dict(
                xt=xt, wmat=wmat, bias=bias, ones1=ones1, iota=iota,
                m_dst=m["m_dst"], m_w1=m["m_w1"], m_w2=m["m_w2"],
                idx_a=m["idx_a"], idx_b=m["idx_b"],
            )
        )
    return in_maps


def kernel(x, W, b, src, dst):
    x = np.asarray(x, np.float32)
    W = np.asarray(W, np.float32)
    b = np.asarray(b, np.float32)
    src = np.asarray(src, np.int32)
    dst = np.asarray(dst, np.int32)

    layout, metas = _build_layout(src, dst)
    key = _layout_key(layout)
    if key not in _PROGRAM_CACHE:
        _PROGRAM_CACHE[key] = _build_program(layout)
    nc = _PROGRAM_CACHE[key]
    in_maps = _build_in_maps(x, W, b, metas)

    from concourse.bass_utils import run_bass_kernel_spmd

    res = run_bass_kernel_spmd(nc, in_maps, list(range(NCORES)))
    h = np.concatenate([res.results[c]["h_out"] for c in range(NCORES)], axis=0)
    mh = np.concatenate([res.results[c]["mh_out"] for c in range(NCORES)], axis=0)
    return (h, mh)
